# revision 66
# baseline (speedup 1.0000x reference)
"""Multi-head attention (B=8, N=1024, C=768, H=12) on 8 Trainium2 NeuronCores.

Sharding: data-parallel over batch — one batch element per core, no collectives.

Per-core dataflow (v2 — attn@V reoriented to halve its PE column count):
  - All matmul operands are bf16 (fp32 PSUM accumulation); fp32 only for
    bias/psum/normalization. Halves DMA and avoids the fp32r <256-col penalty.
  - Q^T,K^T in [o, n] layout (o on partitions); V in [m, o] layout (plain
    h-major head columns, no augmentation).
  - S^T[m, n] = K^T.T @ Q^T per head (contraction over d=64 on partitions).
  - P^T = exp(0.125 * S^T) on ScalarE, bf16 out (no max-subtraction:
    logits ~ N(0,1)).
  - attn@V in [n, d] orientation: out[n, d] (+= over m-tiles) with the P^T
    128x128 chunk as the *stationary* operand and V[m-tile, head] as the
    64-wide moving operand — 64 cols/m-tile instead of 1024: ~half the PE
    columns of the [d, n] orientation. Softmax row-sums from extra 1-col
    matmuls against a ones vector (free in the cost model).
  - normalize on DVE: one reciprocal + one broadcast-multiply per head,
    writing o_big[n, pair, nt, c] bf16.
  - per-pair DMA-engine transpose (InstDmaTransposeAnt) of o_big pair slab
    [128n x 1024(nt,c)] -> oT[c, pair, nt, n]: zero PE/DVE cost.
  - proj y[n, c'] = sum_cb oT_cb.T @ pT_cb + bias, split into a k=0..4
    partial (overlapped with the last pair) and a k=5 finish.
"""

import numpy as np

_STATE = {}

B, N, C = 8, 1024, 768
H, D = 12, 64
KT = 6           # contraction tiles of 128 over C
P = 128
NT = N // P      # 8 n-tiles
PAIRS = H // 2   # 6 head pairs


def _patch_tile_drain():
    """This walrus build rejects >1 sem wait on a CTRL (Drain) instruction.

    TileContext's exit puts one wait per outstanding semaphore on the final SP
    Drain; redistribute them across single-wait NOPs preceding the drain.
    """
    import bass_rust
    import concourse.tile as tile
    from concourse.vector_clock import ScopedClock

    if getattr(tile.TileContext, "_ant_drain_patched", False):
        return

    SyncInfo = bass_rust.SyncInfo

    def _drain_and_barrier(self, tick_clock, wait_clock):
        nc = self.nc
        probe = nc.sync.nop(nofuse=True)
        wait_clock.add_sem_waits(
            probe.ins, ScopedClock({None: tick_clock.global_clock})
        )
        si = probe.ins.sync_info
        waits = list(si.on_wait or []) if si is not None else []
        updates = list(si.on_update or []) if si is not None else []
        if len(waits) > 1:
            probe.ins.sync_info = SyncInfo(on_wait=waits[:1], on_update=updates)
            for w in waits[1:]:
                extra = nc.sync.nop(nofuse=True)
                extra.ins.sync_info = SyncInfo(on_wait=[w], on_update=[])
        nc.sync.drain()

        nc.all_engine_barrier()
        assert self.sems is not None
        popped = nc._tile_sem_poison_stack.pop()
        assert popped is self._sem_poison
        nc.clear_and_free_semaphores(list(self.sems.allocated().values()))
        nc.all_engine_barrier()

    tile.TileContext._drain_and_barrier = _drain_and_barrier
    tile.TileContext._ant_drain_patched = True


def _split_multi_waits(nc):
    """This walrus build allows at most ONE sem wait per instruction.

    Tile's wait assignment routinely puts several; hoist all but the last onto
    single-wait NOPs inserted immediately before the instruction on the same
    engine (engines execute block instructions in order, so semantics are
    unchanged).
    """
    from concourse import mybir

    for fn in nc.m.functions:
        for bb in fn.blocks:
            out, changed = [], False
            for inst in bb.instructions:
                si = inst.sync_info
                waits = list(si.on_wait) if (si is not None and si.on_wait) else []
                if len(waits) > 1:
                    changed = True
                    for w in waits[:-1]:
                        nop = mybir.InstNoOp(
                            name=f"I-ws{nc.next_id()}",
                            engine=inst.engine,
                            bass_nofuse=True,
                            sync_info=mybir.SyncInfo(on_wait=[w], on_update=[]),
                        )
                        nc.register_instruction(nop)
                        out.append(nop)
                    inst.sync_info = mybir.SyncInfo(
                        on_wait=[waits[-1]], on_update=list(si.on_update or [])
                    )
                out.append(inst)
            if changed:
                bb.instructions = out


def _build_nc(trace_sim=False, debug=False):
    from contextlib import ExitStack

    import concourse.bass as bass
    import concourse.tile as tile
    from concourse import mybir

    _patch_tile_drain()

    f32 = mybir.dt.float32
    bf16 = mybir.dt.bfloat16

    nc = bass.Bass("TRN2", target_bir_lowering=False, debug=False, num_devices=1)

    xT = nc.dram_tensor("xT", [KT, P, N], bf16, kind="ExternalInput").ap()
    wqk = nc.dram_tensor("wqk", [PAIRS, P, KT * 256], bf16, kind="ExternalInput").ap()
    wv = nc.dram_tensor("wv", [P, KT, C], bf16, kind="ExternalInput").ap()
    pT = nc.dram_tensor("pT", [P, KT, C], bf16, kind="ExternalInput").ap()
    bias = nc.dram_tensor("bias", [P, C], f32, kind="ExternalInput").ap()
    y = nc.dram_tensor("y", [N, C], f32, kind="ExternalOutput").ap()
    if debug:
        dbg = {
            "dq": nc.dram_tensor("dq", [P, N], bf16, kind="ExternalOutput").ap(),
            "dk": nc.dram_tensor("dk", [P, N], bf16, kind="ExternalOutput").ap(),
            "dv": nc.dram_tensor("dv", [P, C], bf16, kind="ExternalOutput").ap(),
            "drs": nc.dram_tensor("drs", [P, H * NT], f32, kind="ExternalOutput").ap(),
            "dob": nc.dram_tensor(
                "dob", [P, PAIRS * NT * P], bf16, kind="ExternalOutput"
            ).ap(),
            "dot": nc.dram_tensor(
                "dot", [P, PAIRS * NT * P], bf16, kind="ExternalOutput"
            ).ap(),
        }

    Exp = mybir.ActivationFunctionType.Exp
    SCALE = float(D) ** -0.5

    with tile.TileContext(nc, trace_sim=trace_sim) as tc, ExitStack() as ctx:
        kilo = ctx.enter_context(tc.tile_pool(name="kilo", bufs=12))     # xT
        qkp = ctx.enter_context(tc.tile_pool(name="qk", bufs=8))
        wqkp = ctx.enter_context(tc.tile_pool(name="wqk", bufs=3))
        wvp = ctx.enter_context(tc.tile_pool(name="wv", bufs=6))         # wv k-tiles
        bigp = ctx.enter_context(tc.tile_pool(name="big", bufs=1))       # pT
        vp = ctx.enter_context(tc.tile_pool(name="v", bufs=8))
        ptp = ctx.enter_context(tc.tile_pool(name="pt", bufs=18))
        obp = ctx.enter_context(tc.tile_pool(name="ob", bufs=5))         # o_big
        ob5p = ctx.enter_context(tc.tile_pool(name="ob5", bufs=8))
        otp = ctx.enter_context(tc.tile_pool(name="ot", bufs=5))         # oT
        ot5p = ctx.enter_context(tc.tile_pool(name="ot5", bufs=8))
        rsp = ctx.enter_context(tc.tile_pool(name="rs", bufs=1))
        accp = ctx.enter_context(tc.tile_pool(name="acc", bufs=8))
        onep = ctx.enter_context(tc.tile_pool(name="one", bufs=1))
        ps_s = ctx.enter_context(tc.tile_pool(name="pss", bufs=2, space="PSUM"))
        ps_acc = ctx.enter_context(tc.tile_pool(name="psa", bufs=2, space="PSUM"))
        ps_row = ctx.enter_context(tc.tile_pool(name="psr", bufs=1, space="PSUM"))
        ps_misc = ctx.enter_context(tc.tile_pool(name="psm", bufs=1, space="PSUM"))

        # warm the ACT exp table set while input DMAs run (the first real exp
        # otherwise pays the ~2.7us ACT_TABLE_LOAD on the critical path)
        warm = onep.tile([1, 4], f32)
        nc.vector.memset(warm[:], 0.0)
        warm2 = onep.tile([1, 4], f32)
        nc.scalar.activation(warm2[:], warm[:], Exp)

        # ---- load constants / inputs ----
        wq_tiles = {}

        def prefetch_wq(t):
            if t not in wq_tiles:
                wq_t = wqkp.tile([P, KT * 256], bf16, tag="wqk", name=f"wq_{t}")
                if t == 0:
                    # split so the k=0 weight slice (first matmul) lands first
                    nc.sync.dma_start(wq_t[:, 0:256], wqk[0][:, 0:256])
                    nc.sync.dma_start(wq_t[:, 256:], wqk[0][:, 256:])
                else:
                    nc.sync.dma_start(wq_t[:], wqk[t])
                wq_tiles[t] = wq_t

        # x halves as separate tiles: DMA writes track at tile granularity,
        # so a shared tile would false-couple first-half readers to the
        # second-half loads
        prefetch_wq(0)
        xs_a, xs_b = [], []
        for k in range(KT):
            t = kilo.tile([P, 512], bf16, tag="kilo", name=f"xa_{k}")
            eng = nc.scalar if k % 2 == 0 else nc.gpsimd
            eng.dma_start(t[:], xT[k][:, 0:512])
            xs_a.append(t)
        for k in range(KT):
            t = kilo.tile([P, 512], bf16, tag="kilo", name=f"xb_{k}")
            eng = (nc.sync, nc.scalar, nc.gpsimd)[k % 3]
            eng.dma_start(t[:], xT[k][:, 512:1024])
            xs_b.append(t)
        xs_h = (xs_a, xs_b)

        prefetch_wq(1)
        prefetch_wq(2)
        wv_k = []
        for k in range(KT):
            t = wvp.tile([P, C], bf16, tag="wv", name=f"wv_{k}")
            eng = nc.sync if k % 2 == 0 else nc.gpsimd
            eng.dma_start(t[:], wv[:, k, :])
            wv_k.append(t)

        ones_sb = onep.tile([P, 1], bf16)
        nc.vector.memset(ones_sb[:], 1.0)
        bias_sb = onep.tile([P, C], f32)
        nc.gpsimd.dma_start(bias_sb[:], bias[:])

        qt_sb, kt_sb = [], []

        def emit_qk_one(t, which, store):
            """Q^T/K^T of pair t as two half-tiles [128 o, 512 n] bf16."""
            wq_t = wq_tiles[t]
            slot = ps_s.tile([P, 1024], f32, tag="pss", name=f"qk_{t}_{which}")
            halves = []
            for ns in range(2):
                dst = slot[:, ns * 512 : (ns + 1) * 512]
                for k in range(KT):
                    nc.tensor.matmul(
                        dst,
                        wq_t[:, k * 256 + which * P : k * 256 + (which + 1) * P],
                        xs_h[ns][k][:],
                        start=(k == 0),
                        stop=(k == KT - 1),
                    )
                h_t = qkp.tile(
                    [P, 512], bf16, tag="qk", name=f"qk_{t}_{which}_{ns}"
                )
                nc.vector.tensor_copy(h_t[:], dst)
                halves.append(h_t)
            store.append(halves)

        def emit_qk_half(t, which, half, store):
            """Half of a Q^T/K^T tile via the single-bank misc slot."""
            wq_t = wq_tiles[t]
            slot = ps_misc.tile([P, 512], f32, tag="psm", name=f"qkh_{t}_{which}_{half}")
            for k in range(KT):
                nc.tensor.matmul(
                    slot[:],
                    wq_t[:, k * 256 + which * P : k * 256 + (which + 1) * P],
                    xs_h[half][k][:],
                    start=(k == 0),
                    stop=(k == KT - 1),
                )
            h_t = qkp.tile(
                [P, 512], bf16, tag="qk", name=f"qk_{t}_{which}_{half}"
            )
            nc.vector.tensor_copy(h_t[:], slot[:])
            if half == 0:
                store.append([h_t])
            else:
                store[-1].append(h_t)

        # ---- V in [m, o] layout (plain), emitted upfront through ps_s ----
        v_sb = []

        def emit_v(nt):
            slot = ps_s.tile([P, 1024], f32, tag="pss", name=f"v_{nt}")
            xh = xs_h[nt // 4]
            c0x = (nt % 4) * P
            for c0, w in ((0, 512), (512, 256)):
                for k in range(KT):
                    nc.tensor.matmul(
                        slot[:, c0 : c0 + w],
                        xh[k][:, c0x : c0x + P],
                        wv_k[k][:, c0 : c0 + w],
                        start=(k == 0),
                        stop=(k == KT - 1),
                    )
            vt = vp.tile([P, C], bf16, tag="v")
            nc.vector.tensor_copy(vt[:], slot[:, 0:C])
            v_sb.append(vt)

        # pair-0 QK first so S/exp can start early, then all of V emitted
        # back-to-back (pipelines across the two ps_s buffers)
        emit_qk_one(0, 0, qt_sb)
        emit_qk_one(0, 1, kt_sb)
        for j in range(NT):
            emit_v(j)

        pt_w = bigp.tile([P, KT, C], bf16, tag="big")
        nc.gpsimd.dma_start(pt_w[:], pT[:])

        # persistent small tiles, split per pair (tile-granular dep tracking:
        # one shared tile would serialize readers on every later transpose)
        o_bigs = [
            obp.tile([P, NT, P], bf16, tag="ob", name=f"ob_{t}")
            for t in range(PAIRS - 1)
        ]  # [n, nt, c] per pair
        ob5 = [
            ob5p.tile([P, P], bf16, tag="ob5", name=f"ob5_{i}") for i in range(NT)
        ]
        oTs = [
            otp.tile([P, NT, P], bf16, tag="ot", name=f"oT_{t}")
            for t in range(PAIRS - 1)
        ]  # [c, nt, n] per pair
        oT5 = [
            ot5p.tile([P, P], bf16, tag="ot5", name=f"oT5_{i}") for i in range(NT)
        ]
        rs_sb = rsp.tile([P, H * NT], f32, tag="rs")            # 1/rowsum
        row_ps = ps_row.tile([P, H * NT], f32, tag="psr")       # rowsums

        acc_sb = {}

        def proj_stage(nt, cb0, cb1):
            """Accumulate proj k-tiles [cb0, cb1) for n-tile nt into SBUF."""
            if nt not in acc_sb:
                acc_sb[nt] = accp.tile([P, C], f32, tag="acc", name=f"acc_{nt}")
            acc = acc_sb[nt]
            for c0, w in ((0, 512), (512, 256)):
                slot = ps_misc.tile([P, w], f32, tag="psm", name=f"pp_{nt}_{cb0}_{c0}")
                for cb in range(cb0, cb1):
                    nc.tensor.matmul(
                        slot[:],
                        oTs[cb][:, nt, :],
                        pt_w[:, cb, c0 : c0 + w],
                        start=(cb == cb0),
                        stop=(cb == cb1 - 1),
                        skip_group_check=True,
                    )
                if cb0 == 0:
                    nc.vector.tensor_add(
                        acc[:, c0 : c0 + w], slot[:], bias_sb[:, c0 : c0 + w]
                    )
                else:
                    nc.vector.tensor_add(
                        acc[:, c0 : c0 + w], acc[:, c0 : c0 + w], slot[:]
                    )

        def proj_finish(nt):
            """k=4,5 + add + store for n-tile nt (ps_s banks are free by now).

            cb4 matmuls emitted first: they don't depend on the last pair's
            transpose, so PE can run them while the transpose DMA lands."""
            slot = ps_s.tile([P, 1024], f32, tag="pss", name=f"pf_{nt}")
            for cb in (KT - 2, KT - 1):
                src = oTs[cb][:, nt, :] if cb < KT - 1 else oT5[nt][:]
                for c0, w in ((0, 512), (512, 256)):
                    nc.tensor.matmul(
                        slot[:, c0 : c0 + w],
                        src,
                        pt_w[:, cb, c0 : c0 + w],
                        start=(cb == KT - 2),
                        stop=(cb == KT - 1),
                        skip_group_check=True,
                    )
            acc = acc_sb.pop(nt)
            nc.vector.tensor_add(acc[:], acc[:], slot[:, 0:C])
            eng = nc.scalar if nt % 2 == 0 else nc.sync
            eng.dma_start(y[nt * P : (nt + 1) * P, :], acc[:])

        # ---- attention ----
        # PSUM banks allow only ONE open accumulation group at a time (a
        # start=True resets the bank's accumulation context), so the m-loop
        # of each (head, nt) output group must run back-to-back. Structure:
        # software pipeline with head-slots — slot s computes S+exp of head s
        # while running attn@V of head s-1 against its 8 retained P^T tiles.
        pt_tiles = {}

        def phase1(h, j):
            t, hb = h // 2, (h % 2) * D
            s_slot = ps_s.tile([P, 1024], f32, tag="pss", name=f"s_{h}_{j}")
            kt_h = kt_sb[t][j // 4]
            for ns in range(2):
                nc.tensor.matmul(
                    s_slot[:, ns * 512 : (ns + 1) * 512],
                    kt_h[hb : hb + D, (j % 4) * P : (j % 4 + 1) * P],
                    qt_sb[t][ns][hb : hb + D, :],
                    start=True,
                    stop=True,
                )
            pt_t = ptp.tile([P, 1024], bf16, tag="pt", name=f"pt_{h}_{j}")
            nc.scalar.activation(pt_t[:], s_slot[:], Exp, scale=SCALE)
            pt_tiles.setdefault(h, {})[j] = pt_t

        def phase2(h, nt, acc_ap, acc_c, row_ap, row_c):
            pts = pt_tiles[h]
            for j in range(NT):
                nc.tensor.matmul(
                    acc_ap[:, acc_c : acc_c + D],
                    pts[j][:, nt * P : (nt + 1) * P],
                    v_sb[j][:, h * D : (h + 1) * D],
                    start=(j == 0),
                    stop=(j == NT - 1),
                    skip_group_check=True,
                )
            for j in range(NT):
                nc.tensor.matmul(
                    row_ap[:, row_c : row_c + 1],
                    pts[j][:, nt * P : (nt + 1) * P],
                    ones_sb[:, 0:1],
                    start=(j == 0),
                    stop=(j == NT - 1),
                    skip_group_check=True,
                )

        def normalize(h, acc):
            t, hb = h // 2, (h % 2) * D
            nc.vector.reciprocal(
                rs_sb[:, h * NT : (h + 1) * NT], row_ps[:, h * NT : (h + 1) * NT]
            )
            rs_base = rs_sb[:, h * NT : (h + 1) * NT]
            rs_bcast = bass.AP(
                tensor=rs_base.tensor,
                offset=rs_base.offset,
                ap=[list(rs_base.ap[0]), list(rs_base.ap[1]), [0, D]],
            )
            nc.vector.tensor_mul(
                o_bigs[t][:, :, hb : hb + D], acc[:, 0:512], rs_bcast
            )

        accs = {}
        for slot in range(H + 1):
            h1 = slot        # phase-1 head (S + exp)
            h2 = slot - 1    # phase-2 head (attn@V + rowsums)
            # both heads of the last pair run the per-nt endgame pattern
            last2 = h2 >= H - 2
            if h2 >= 0 and not last2:
                accs[h2] = ps_acc.tile([P, 512], f32, tag="psa", name=f"o_{h2}")
            if last2:
                # two banks per head so per-nt normalization can interleave
                # with accumulation (start=True resets a bank's accumulation
                # context, forcing whole-bank WAR ordering); rowsums go in
                # the same banks (col 256+) instead of the shared row bank
                accs[h2] = [
                    ps_acc.tile([P, 512], f32, tag="psa", name=f"oe{h2}a"),
                    ps_acc.tile([P, 512], f32, tag="psa", name=f"oe{h2}b"),
                ]
            for step in range(NT):
                if h1 < H:
                    phase1(h1, step)
                if h2 >= 0 and not last2:
                    phase2(h2, step, accs[h2], step * D,
                           row_ps, h2 * NT + step)
                if last2:
                    hb2 = (h2 % 2) * D
                    a_t = accs[h2][step % 2]
                    g = step // 2
                    phase2(h2, step, a_t, g * D, a_t, 256 + g)
                    c = h2 * NT + step
                    nc.vector.reciprocal(
                        rs_sb[:, c : c + 1], a_t[:, 256 + g : 257 + g]
                    )
                    if h2 == H - 1:
                        # the last head's normalize runs on the (now idle)
                        # ACT engine as a scale-activation, freeing DVE for
                        # the projection-finish adds
                        nc.scalar.activation(
                            ob5[step][:, hb2 : hb2 + D],
                            a_t[:, g * D : (g + 1) * D],
                            mybir.ActivationFunctionType.Copy,
                            scale=rs_sb[:, c : c + 1],
                        )
                        # per-nt transpose + projection finish chase the
                        # normalized tiles (alternate dispatch queues)
                        teng = nc.sync if step % 2 == 0 else nc.scalar
                        teng.dma_start_transpose(oT5[step][:], ob5[step][:])
                        proj_finish(step)
                    else:
                        nc.vector.tensor_scalar_mul(
                            ob5[step][:, hb2 : hb2 + D],
                            a_t[:, g * D : (g + 1) * D],
                            rs_sb[:, c : c + 1],
                        )
                # upcoming QK emission through the misc bank: pair p's
                # q halves in slot 2p-2 (steps 1,4), k halves in slot 2p-1
                # (steps 0,3)
                if slot <= 9:
                    p = slot // 2 + 1
                    if slot % 2 == 0 and step == 1:
                        emit_qk_half(p, 0, 0, qt_sb)
                    elif slot % 2 == 0 and step == 4:
                        emit_qk_half(p, 0, 1, qt_sb)
                    elif slot % 2 == 1 and step == 0:
                        emit_qk_half(p, 1, 0, kt_sb)
                    elif slot % 2 == 1 and step == 3:
                        emit_qk_half(p, 1, 1, kt_sb)
                if slot in (2, 4, 6) and step == 6:
                    prefetch_wq(slot // 2 + 2)
                # projection partials through the misc bank, staged as their
                # k-tiles become available (pair p transposed at slot 2p+2):
                # cb0-1 in slots 5-8, cb2-3 in slots 9-11
                if slot in (5, 6, 7, 8) and step in (2, 5):
                    proj_stage((slot - 5) * 2 + step // 4, 0, 2)
                elif slot == 9 and step in (5, 7):
                    proj_stage((step - 5) // 2, 2, 4)
                elif slot == 10 and step in (1, 3, 5, 7):
                    proj_stage(2 + step // 2, 2, 4)
                elif slot == 11 and step in (1, 3):
                    proj_stage(6 + (step - 1) // 2, 2, 4)
            if h2 >= 0 and not last2:
                normalize(h2, accs.pop(h2))
                del pt_tiles[h2]
                if h2 % 2 == 1:
                    t = h2 // 2
                    nc.sync.dma_start_transpose(oTs[t][:], o_bigs[t][:])
            elif h2 >= 0:
                del pt_tiles[h2]

        if debug:
            for ns in range(2):
                nc.sync.dma_start(
                    dbg["dq"][:, ns * 512 : (ns + 1) * 512], qt_sb[0][ns][:]
                )
                nc.sync.dma_start(
                    dbg["dk"][:, ns * 512 : (ns + 1) * 512], kt_sb[0][ns][:]
                )
            nc.sync.dma_start(dbg["dv"][:], v_sb[0][:])
            nc.sync.dma_start(dbg["drs"][:], rs_sb[:])
            for t in range(PAIRS - 1):
                nc.sync.dma_start(
                    dbg["dob"][:, t * N : (t + 1) * N],
                    o_bigs[t].rearrange("p a b -> p (a b)"),
                )
                nc.sync.dma_start(
                    dbg["dot"][:, t * N : (t + 1) * N],
                    oTs[t].rearrange("p a b -> p (a b)"),
                )
            for i in range(NT):
                o5 = (PAIRS - 1) * N + i * P
                nc.sync.dma_start(dbg["dob"][:, o5 : o5 + P], ob5[i][:])
                nc.sync.dma_start(dbg["dot"][:, o5 : o5 + P], oT5[i][:])

    _split_multi_waits(nc)
    return nc


def _prep_shared(qkv_w, proj_w, proj_b):
    import ml_dtypes

    bf = ml_dtypes.bfloat16
    f = np.float32
    wqT = qkv_w[0:C].astype(f).T.copy()           # [c, o]
    wkT = qkv_w[C : 2 * C].astype(f).T.copy()
    wvT = qkv_w[2 * C : 3 * C].astype(f).T.copy()

    wqk = np.zeros((PAIRS, P, KT, 256), f)
    for t in range(PAIRS):
        for k in range(KT):
            wqk[t, :, k, 0:P] = wqT[k * P : (k + 1) * P, t * P : (t + 1) * P]
            wqk[t, :, k, P:256] = wkT[k * P : (k + 1) * P, t * P : (t + 1) * P]
    wqk = wqk.reshape(PAIRS, P, KT * 256).astype(bf)

    wvh = wvT.reshape(KT, P, C).transpose(1, 0, 2).astype(bf).copy()
    pTh = proj_w.T.astype(f).reshape(KT, P, C).transpose(1, 0, 2).astype(bf).copy()
    bias_h = np.ascontiguousarray(np.broadcast_to(proj_b.astype(f), (P, C)))
    return wqk, wvh, pTh, bias_h


def kernel(x, qkv_w, proj_w, proj_b):
    import ml_dtypes
    from concourse.bass_utils import run_bass_kernel_spmd

    bf = ml_dtypes.bfloat16
    x = np.asarray(x, np.float32)
    wqk, wvh, pTh, bias_h = _prep_shared(
        np.asarray(qkv_w), np.asarray(proj_w), np.asarray(proj_b)
    )

    if "nc" not in _STATE:
        _STATE["nc"] = _build_nc()
    nc = _STATE["nc"]

    in_maps = []
    for b in range(B):
        xTb = np.ascontiguousarray(x[b].T).reshape(KT, P, N).astype(bf)
        in_maps.append(
            {"xT": xTb, "wqk": wqk, "wv": wvh, "pT": pTh, "bias": bias_h}
        )

    res = run_bass_kernel_spmd(nc, in_maps, core_ids=list(range(B)))
    return np.stack([res.results[b]["y"] for b in range(B)], axis=0)
